# revision 25
# baseline (speedup 1.0000x reference)
"""Trainium2 Bass kernel for nn_NonLocalNd_bn_cbam (non-local attention + BN
whitening + global-context branch), data-parallel over batch on 8 NeuronCores.

Hardcoded problem shape: x [8, 256, 64, 64], P=128 projections, maxpool2x2 for
k/v (Nk=1024), Nq=4096.  Each core handles one batch element; the only
cross-core coupling is the BatchNorm whitening statistics of q and k, handled
by a tiny [128, 4] AllReduce.

Math restructuring (softmax over the key axis m is invariant to per-query
terms, so only the K side of the whitening has to materialize):
  - spatial whitening (subtract channel-mean) folded into w_q/w_k on the host.
  - kn = w * (k - mk) with w = rstd_q*rstd_k = rsqrt((var_q+eps)(var_k+eps));
    q stays RAW (its whitening collapses into a per-key bias
    T3[m] = sum_p mean_q[p]*kn[p,m], applied inside the exp as a
    per-partition activation bias).  The query-side mean/std terms are
    per-query constants that cancel in the softmax.
  - the whole kernel runs on a single fp16 copy of x (projections, maxpool,
    attention operands, and the residual all read it; no separate fp32 load).
  - softmax denominator: e-chunks accumulated on the Vector engine, then a
    single ones-vector matmul per block; gamma folded into the ones vector.
  - global-context branch folded into the output projection residual via
    xgc = w_out @ gc (per-partition scalar in the flush fused-add).

Schedule: x loads stream in chunk-by-chunk with maxpool / q-proj / bn_stats
pipelined behind them; the stats AllReduce fires as soon as the k stats are
aggregated; the v/mask/gc branch and act-table warmup run inside the
collective's latency window; attention runs with av lagging sim by one block
so the PE never waits on the exps.
"""

import math

import ml_dtypes
import numpy as np

import concourse.bass as bass
import concourse.mybir as mybir
import concourse.tile as tile
from concourse import bacc
from concourse.bass_isa import ReduceOp
from concourse.bass_utils import run_bass_kernel_spmd

F32 = mybir.dt.float32
F16 = mybir.dt.float16
BF16 = mybir.dt.bfloat16
U32 = mybir.dt.uint32
AF = mybir.ActivationFunctionType
OP = mybir.AluOpType
AX = mybir.AxisListType

B, CIN, H, W = 8, 256, 64, 64
P = 128
NQ = H * W                # 4096
NK = (H // 2) * (W // 2)  # 1024
N_CORES = 8
EPS = 1e-5
INV_SCALE = 1.0 / math.sqrt(P)   # temperature 1.0

LAST_RESULTS = None  # test harness reads exec_time from here


def _maybe_shim_trace_hooks():
    """If BASS_TRACE is set in the environment, bass_utils imports
    antenv.axon_hooks, which this container image lacks.  Recreate it (and
    stub the artifact upload) so tracing degrades gracefully instead of
    crashing; a failure here is harmless for the non-traced path."""
    import os
    import sys
    import types

    if not os.environ.get("BASS_TRACE"):
        return
    try:
        import antenv.axon_hooks  # noqa: F401
        return
    except ImportError:
        pass
    try:
        import antenv
        from trn_agent_boot.trn_boot import _ntff_profile_via_ctypes

        hook = _ntff_profile_via_ctypes("/opt/axon/libaxon_pjrt.so")
        m = types.ModuleType("antenv.axon_hooks")
        m.get_axon_ntff_profile_hook = lambda: hook
        m.set_axon_ntff_profile_hook = lambda h: None
        sys.modules["antenv.axon_hooks"] = m
        antenv.axon_hooks = m
        from concourse import bass_utils as _bu

        _bu.upload_artifacts = lambda tmpdir: tmpdir
    except Exception:
        os.environ["BASS_NEVER_TRACE"] = "1"


def _build_bass(inv_gamma: float):
    nc = bacc.Bacc("TRN2", target_bir_lowering=False)

    # ---- per-core I/O ----------------------------------------------------
    x_d = nc.dram_tensor("x", [CIN, NQ], F16, kind="ExternalInput")
    # packed fp32 q/k weights: [2, 128, 256] = (wqT | wkT) per cin chunk
    wqk_d = nc.dram_tensor("wqk", [2, 128, 256], F16, kind="ExternalInput")
    # packed fp32 v/mask weights: [2, 128, 129] = (wvT | wmT) per cin chunk
    wvm_d = nc.dram_tensor("wvm", [2, 128, 129], F16, kind="ExternalInput")
    bqk_d = nc.dram_tensor("bqk", [P, 2], F32, kind="ExternalInput")  # bq|bk
    bvm_d = nc.dram_tensor("bvm", [1, 129], F32, kind="ExternalInput")  # bv|0
    woutT_d = nc.dram_tensor("woutT", [P, CIN], F16, kind="ExternalInput")
    out_d = nc.dram_tensor("out", [CIN, NQ], F32, kind="ExternalOutput")

    groups = [list(range(N_CORES))]

    with tile.TileContext(nc) as tc:
        with (
            tc.tile_pool(name="consts", bufs=1) as consts,
            tc.tile_pool(name="bigs", bufs=1) as bigs,
            tc.tile_pool(name="mp", bufs=4) as mp,
            tc.tile_pool(name="small", bufs=1) as small,
            tc.tile_pool(name="dram", bufs=1, space="DRAM") as dramp,
        ):
            cc_in_d = dramp.tile([P, 4], F32, tag="cc_in")
            cc_out_d = dramp.tile([P, 4], F32, tag="cc_out", addr_space="Shared")

            # ---- input loads (x fp32 once; chunk-interleaved) -----------
            x_sb = [bigs.tile([128, NQ], F16, name=f"x{ct}", tag=f"x{ct}") for ct in range(2)]
            for j in range(4):
                for ct in range(2):
                    nc.sync.dma_start(
                        out=x_sb[ct][:, j * 1024:(j + 1) * 1024],
                        in_=x_d[ct * 128:(ct + 1) * 128, j * 1024:(j + 1) * 1024],
                    )
            wqk_t = consts.tile([128, 2, 256], F16, tag="wqk")
            wvm_t = consts.tile([128, 2, 129], F16, tag="wvm")
            bqk_t = consts.tile([128, 2], F32, tag="bqk")
            bvm_row = consts.tile([1, 129], F32, tag="bvmrow")
            wout_t = consts.tile([128, CIN], F16, tag="wout")
            # q/k weights + biases on the Act queue (needed first);
            # v/mask/out weights on the cheap-to-issue Pool queue.
            for cc in range(2):
                nc.scalar.dma_start(out=wqk_t[:, cc, :], in_=wqk_d[cc, :, :])
            nc.scalar.dma_start(out=bqk_t, in_=bqk_d[:, :])
            for cc in range(2):
                nc.gpsimd.dma_start(out=wvm_t[:, cc, :], in_=wvm_d[cc, :, :])
            nc.gpsimd.dma_start(out=bvm_row, in_=bvm_d[:, :])
            nc.gpsimd.dma_start(out=wout_t, in_=woutT_d[:, :])

            def wq(cc):
                return wqk_t[:, cc, 0:128]

            def wk(cc):
                return wqk_t[:, cc, 128:256]

            # ---- act-table warmup: exp early, anchored via ones_ig -------
            warm = small.tile([128, 1], F32, tag="warm")
            nc.vector.memset(warm, 0.0)
            warm_exp = small.tile([128, 1], F32, tag="warm_exp")
            nc.scalar.activation(warm_exp, warm, AF.Exp)
            ones_ig = consts.tile([128, 1], F16, tag="ones_ig")

            # ---- maxpool (fp32, chunk-pipelined behind the x DMAs) -------
            xp_sb = [bigs.tile([128, NK], F16, name=f"xp{ct}", tag=f"xp{ct}") for ct in range(2)]
            q_sb = bigs.tile([128, NQ], F16, tag="q")
            k_sb = bigs.tile([128, NK], F16, tag="k")
            kn_sb = bigs.tile([128, NK], F16, tag="kn")
            stats_q = small.tile([128, 8, 6], F32, tag="stats_q")
            stats_k = small.tile([128, 2, 6], F32, tag="stats_k")
            vT = bigs.tile([128, 8, 129], F16, tag="vT")
            e_big = [bigs.tile([128, 8, 1024], F16, name=f"e{i}", tag=f"e{i}") for i in range(2)]
            eacc = [bigs.tile([128, 1024], F16, name=f"ea{i}", tag=f"ea{i}") for i in range(2)]
            outsim = bigs.tile([128, NQ], F16, tag="outsim")

            with (
                tc.tile_pool(name="ps_qp", bufs=2, space="PSUM") as ps_qp,
                tc.tile_pool(name="ps_kp", bufs=1, space="PSUM") as ps_kp,
                tc.tile_pool(name="ps_vp", bufs=1, space="PSUM") as ps_vp,
                tc.tile_pool(name="ps_g", bufs=1, space="PSUM") as ps_g,
                tc.tile_pool(name="ps_t3", bufs=1, space="PSUM") as ps_t3p,
            ):
                # ---- q projection (fp32r off raw x) + per-chunk stats ----
                for j in range(8):
                    qp = ps_qp.tile([128, 512], F32, name=f"qp{j}", tag="qp")
                    for cc in range(2):
                        nc.tensor.matmul(
                            qp,
                            wq(cc),
                            x_sb[cc][:, j * 512:(j + 1) * 512],
                            start=(cc == 0),
                            stop=(cc == 1),
                        )
                    nc.scalar.activation(
                        q_sb[:, j * 512:(j + 1) * 512], qp, AF.Identity,
                        bias=bqk_t[:, 0:1],
                    )
                    nc.vector.bn_stats(stats_q[:, j, :], q_sb[:, j * 512:(j + 1) * 512])
                    # maxpool of this quarter (j pairs: quarter q = j//2)
                    if j % 2 == 1:
                        qq = j // 2
                        for ct in range(2):
                            xv = x_sb[ct][:, qq * 1024:(qq + 1) * 1024].rearrange(
                                "p (i a j b) -> p i a j b", i=8, a=2, j=32, b=2
                            )
                            t1 = mp.tile([128, 8, 32], F16, name=f"t1_{qq}_{ct}", tag="mp1")
                            t2 = mp.tile([128, 8, 32], F16, name=f"t2_{qq}_{ct}", tag="mp2")
                            xo = xp_sb[ct][:, qq * 256:(qq + 1) * 256].rearrange(
                                "p (i j) -> p i j", i=8
                            )
                            nc.vector.tensor_max(t1, xv[:, :, 0, :, 0], xv[:, :, 0, :, 1])
                            nc.vector.tensor_max(t2, xv[:, :, 1, :, 0], xv[:, :, 1, :, 1])
                            nc.vector.tensor_max(xo, t1, t2)

                # ---- k projection + stats --------------------------------
                kp = ps_kp.tile([128, NK], F32, tag="kp")
                for hh in range(2):
                    for cc in range(2):
                        nc.tensor.matmul(
                            kp[:, hh * 512:(hh + 1) * 512],
                            wk(cc),
                            xp_sb[cc][:, hh * 512:(hh + 1) * 512],
                            start=(cc == 0),
                            stop=(cc == 1),
                        )
                nc.scalar.activation(k_sb, kp, AF.Identity, bias=bqk_t[:, 1:2])
                for hh in range(2):
                    nc.vector.bn_stats(stats_k[:, hh, :], k_sb[:, hh * 512:(hh + 1) * 512])

                # ---- pack local moments, fire AllReduce ASAP -------------
                mv_q = small.tile([128, 2], F32, tag="mv_q")
                mv_k = small.tile([128, 2], F32, tag="mv_k")
                nc.vector.bn_aggr(mv_q, stats_q)
                nc.vector.bn_aggr(mv_k, stats_k)
                cc_sb = small.tile([128, 4], F32, tag="cc_sb")
                tq = small.tile([128, 1], F32, tag="tq")
                nc.vector.tensor_scalar(
                    out=cc_sb[:, 0:1], in0=mv_q[:, 0:1], scalar1=float(NQ),
                    scalar2=None, op0=OP.mult,
                )
                nc.vector.tensor_mul(tq, mv_q[:, 0:1], mv_q[:, 0:1])
                nc.vector.tensor_add(tq, tq, mv_q[:, 1:2])
                nc.vector.tensor_scalar(
                    out=cc_sb[:, 1:2], in0=tq, scalar1=float(NQ),
                    scalar2=None, op0=OP.mult,
                )
                tk = small.tile([128, 1], F32, tag="tk")
                nc.vector.tensor_scalar(
                    out=cc_sb[:, 2:3], in0=mv_k[:, 0:1], scalar1=float(NK),
                    scalar2=None, op0=OP.mult,
                )
                nc.vector.tensor_mul(tk, mv_k[:, 0:1], mv_k[:, 0:1])
                nc.vector.tensor_add(tk, tk, mv_k[:, 1:2])
                nc.vector.tensor_scalar(
                    out=cc_sb[:, 3:4], in0=tk, scalar1=float(NK),
                    scalar2=None, op0=OP.mult,
                )
                nc.gpsimd.dma_start(out=cc_in_d[:, :], in_=cc_sb)
                nc.gpsimd.collective_compute(
                    "AllReduce", OP.add, replica_groups=groups,
                    ins=[cc_in_d.opt()], outs=[cc_out_d.opt()],
                )
                g_sb = small.tile([128, 4], F32, tag="g_sb")
                nc.sync.dma_start(out=g_sb, in_=cc_out_d[:, :])

                # ---- collective-window work: v/mask proj, gc branch ------
                nc.vector.tensor_scalar(
                    out=ones_ig, in0=warm_exp, scalar1=inv_gamma, scalar2=None,
                    op0=OP.mult,
                )
                bvm_bc = consts.tile([128, 129], F32, tag="bvmbc")
                nc.gpsimd.partition_broadcast(bvm_bc, bvm_row, 128)
                for mc in range(8):
                    vp = ps_vp.tile([128, 129], F32, name=f"vp{mc}", tag="vp")
                    for cc in range(2):
                        nc.tensor.matmul(
                            vp,
                            xp_sb[cc][:, mc * 128:(mc + 1) * 128],
                            wvm_t[:, cc, :],
                            start=(cc == 0),
                            stop=(cc == 1),
                        )
                    nc.vector.scalar_tensor_tensor(
                        out=vT[:, mc, :], in0=vp, scalar=1.0, in1=bvm_bc,
                        op0=OP.mult, op1=OP.add,
                    )

                em = small.tile([128, 8], F16, tag="em")
                nc.scalar.activation(em, vT[:, :, 128], AF.Exp)
                s1 = small.tile([128, 1], F32, tag="s1")
                nc.vector.reduce_sum(s1, em, axis=AX.X)
                s_bc = small.tile([128, 1], F32, tag="s_bc")
                nc.gpsimd.partition_all_reduce(s_bc, s1, 128, ReduceOp.add)
                r_s = small.tile([128, 1], F32, tag="r_s")
                nc.vector.reciprocal_approx_fast(out=r_s, in_=s_bc)
                gcp = ps_g.tile([128, 1], F32, tag="gcp")
                for mc in range(8):
                    nc.tensor.matmul(
                        gcp, vT[:, mc, 0:128], em[:, mc:mc + 1],
                        start=(mc == 0), stop=(mc == 7),
                    )
                gc_bf = small.tile([128, 1], F16, tag="gc_bf")
                nc.vector.tensor_scalar(
                    out=gc_bf, in0=gcp, scalar1=r_s, scalar2=None, op0=OP.mult
                )
                # xgc[c] = (w_out @ gc)[c]; folded into the flush residual add
                xgp = ps_g.tile([128, 2], F32, tag="xgp")
                for ct in range(2):
                    nc.tensor.matmul(
                        xgp[:, ct:ct + 1],
                        wout_t[:, ct * 128:(ct + 1) * 128],
                        gc_bf,
                        start=True, stop=True,
                    )
                xgc = small.tile([128, 2], F32, tag="xgc")
                nc.scalar.activation(xgc, xgp, AF.Identity)

                # ---- post-collective: w, kn, exp-bias --------------------
                g_names = {}
                for (sl, inv_n, key) in ((0, 1.0 / (B * NQ), "q"), (2, 1.0 / (B * NK), "k")):
                    gm = small.tile([128, 1], F32, name=f"gm{key}", tag=f"gm{key}")
                    e2 = small.tile([128, 1], F32, name=f"e2{key}", tag=f"e2{key}")
                    veps = small.tile([128, 1], F32, name=f"veps{key}", tag=f"veps{key}")
                    nc.vector.tensor_scalar(
                        out=gm, in0=g_sb[:, sl:sl + 1], scalar1=inv_n,
                        scalar2=None, op0=OP.mult,
                    )
                    nc.vector.tensor_scalar(
                        out=e2, in0=g_sb[:, sl + 1:sl + 2], scalar1=inv_n,
                        scalar2=None, op0=OP.mult,
                    )
                    # veps = e2 - gm^2 + EPS
                    nc.vector.tensor_mul(veps, gm, gm)
                    nc.vector.tensor_sub(veps, e2, veps)
                    nc.vector.tensor_scalar(
                        out=veps, in0=veps, scalar1=EPS, scalar2=None, op0=OP.add,
                    )
                    g_names[key] = (gm, veps)
                gmq, vq = g_names["q"]
                gmk, vk = g_names["k"]
                prod = small.tile([128, 1], F32, tag="prod")
                nc.vector.tensor_mul(prod, vq, vk)
                # w = rsqrt(prod): integer-seeded Newton on the DVE only —
                # avoids Ln/Exp act-table reloads on the post-collective path
                c5f = small.tile([128, 1], U32, tag="c5f")
                nc.vector.memset(c5f, 0x5F3759DF)
                tsh = small.tile([128, 1], U32, tag="tsh")
                nc.vector.tensor_scalar(
                    out=tsh, in0=prod.bitcast(U32), scalar1=1, scalar2=None,
                    op0=OP.logical_shift_right,
                )
                w_t = small.tile([128, 1], F32, tag="w_t")
                nc.vector.tensor_sub(w_t.bitcast(U32), c5f, tsh)
                nr_a = small.tile([128, 1], F32, tag="nr_a")
                for _ in range(2):
                    nc.vector.tensor_mul(nr_a, w_t, w_t)
                    nc.vector.tensor_mul(nr_a, nr_a, prod)
                    nc.vector.tensor_scalar(
                        out=nr_a, in0=nr_a, scalar1=-0.5, scalar2=1.5,
                        op0=OP.mult, op1=OP.add,
                    )
                    nc.vector.tensor_mul(w_t, w_t, nr_a)
                knb = small.tile([128, 1], F32, tag="knb")
                nc.vector.tensor_scalar(
                    out=knb, in0=gmk, scalar1=w_t, scalar2=-1.0,
                    op0=OP.mult, op1=OP.mult,
                )
                nc.vector.tensor_scalar(
                    out=kn_sb, in0=k_sb, scalar1=w_t, scalar2=knb,
                    op0=OP.mult, op1=OP.add,
                )
                # T3[m] = sum_p gmq[p] * kn[p, m]  (per-key exp bias)
                gmq_h = small.tile([128, 1], F16, tag="gmq_h")
                nc.vector.tensor_copy(gmq_h, gmq)
                t3p = ps_t3p.tile([128, 8], F32, tag="t3")
                for mc in range(8):
                    nc.tensor.matmul(
                        t3p[:, mc:mc + 1],
                        kn_sb[:, mc * 128:(mc + 1) * 128],
                        gmq_h,
                        start=True, stop=True,
                    )
                ebias = small.tile([128, 8], F32, tag="ebias")
                nc.scalar.activation(ebias, t3p, AF.Copy, bias=-2.0, scale=-INV_SCALE)

            # ---- phase 2: attention + fused output projection ------------
            with (
                tc.tile_pool(name="ps_sim", bufs=2, space="PSUM") as ps_sim,
                tc.tile_pool(name="ps_av", bufs=1, space="PSUM") as ps_av,
                tc.tile_pool(name="ps_cs", bufs=1, space="PSUM") as ps_cs,
                tc.tile_pool(name="rows", bufs=2) as rows,
                tc.tile_pool(name="rbcp", bufs=2) as rbcp,
                tc.tile_pool(name="outp", bufs=3) as outp,
            ):
                av_ps = {}
                cs_ps = {}

                def emit_sim(b, mc):
                    sim = ps_sim.tile([128, 1024], F32, name=f"sim{b}_{mc}", tag="sim")
                    for hh in range(2):
                        nc.tensor.matmul(
                            sim[:, hh * 512:(hh + 1) * 512],
                            kn_sb[:, mc * 128:(mc + 1) * 128],
                            q_sb[:, b * 1024 + hh * 512:b * 1024 + (hh + 1) * 512],
                            start=True, stop=True,
                        )
                    eb = e_big[b % 2]
                    nc.scalar.activation(
                        eb[:, mc, :], sim, AF.Exp,
                        bias=ebias[:, mc:mc + 1], scale=INV_SCALE,
                    )
                    ea = eacc[b % 2]
                    if mc == 1:
                        nc.vector.tensor_add(ea, eb[:, 0, :], eb[:, 1, :])
                    elif mc > 1:
                        nc.vector.tensor_add(ea, ea, eb[:, mc, :])

                def emit_av(b, mc):
                    if mc == 0:
                        av_ps[b] = ps_av.tile([128, 1024], F32, name=f"av{b}", tag="av")
                    av = av_ps[b]
                    eb = e_big[b % 2]
                    for hh in range(2):
                        nc.tensor.matmul(
                            av[:, hh * 512:(hh + 1) * 512],
                            vT[:, mc, 0:128],
                            eb[:, mc, hh * 512:(hh + 1) * 512],
                            start=(mc == 0), stop=(mc == 7),
                        )

                rbc_sb = {}

                def emit_cs(b):
                    # colsum + reciprocal + broadcast only — must NOT read
                    # the av psum (its accumulation group may still be open)
                    cs = ps_cs.tile([1, 1024], F32, name=f"cs{b}", tag="cs")
                    cs_ps[b] = cs
                    for hh in range(2):
                        nc.tensor.matmul(
                            cs[:, hh * 512:(hh + 1) * 512], ones_ig,
                            eacc[b % 2][:, hh * 512:(hh + 1) * 512],
                            start=True, stop=True,
                        )
                    rrow = rows.tile([1, 1024], F32, name=f"rr{b}", tag="rrow")
                    nc.vector.reciprocal_approx_fast(out=rrow, in_=cs)
                    rbc = rbcp.tile([128, 1024], F32, name=f"rbc{b}", tag="rbc")
                    nc.gpsimd.partition_broadcast(rbc, rrow, 128)
                    rbc_sb[b] = rbc

                def emit_mul(b):
                    nc.vector.tensor_mul(
                        outsim[:, b * 1024:(b + 1) * 1024], av_ps[b], rbc_sb[b]
                    )

                def emit_flush(b):
                    # out[c, nb] = w_out @ outsim[:, nb] + xgc[c] + x[c, nb]
                    for ct in range(2):
                        op = ps_sim.tile([128, 1024], F32, name=f"op{b}_{ct}", tag="sim")
                        for hh in range(2):
                            nc.tensor.matmul(
                                op[:, hh * 512:(hh + 1) * 512],
                                wout_t[:, ct * 128:(ct + 1) * 128],
                                outsim[:, b * 1024 + hh * 512:b * 1024 + (hh + 1) * 512],
                                start=True, stop=True,
                            )
                        ot = outp.tile([128, 1024], F32, name=f"ot{b}_{ct}", tag="ot")
                        nc.vector.scalar_tensor_tensor(
                            out=ot, in0=op, scalar=xgc[:, ct:ct + 1],
                            in1=x_sb[ct][:, b * 1024:(b + 1) * 1024],
                            op0=OP.add, op1=OP.add,
                        )
                        nc.sync.dma_start(
                            out=out_d[ct * 128:(ct + 1) * 128, b * 1024:(b + 1) * 1024],
                            in_=ot,
                        )

                # steady state: block b-1's av/cs/mul/flush interleave with
                # the first half of block b's sims, closing the av group and
                # draining the flush mid-block so there is no PE bubble at
                # block boundaries (the exps pace the loop).
                for b in range(4):
                    for mc in range(8):
                        emit_sim(b, mc)
                        if b >= 1:
                            if mc == 0:
                                emit_cs(b - 1)
                            if mc < 4:
                                emit_av(b - 1, 2 * mc)
                                emit_av(b - 1, 2 * mc + 1)
                            elif mc == 4:
                                emit_mul(b - 1)
                                emit_flush(b - 1)
                for mc in range(8):
                    emit_av(3, mc)
                emit_cs(3)
                emit_mul(3)
                emit_flush(3)

    nc.compile()
    return nc


def kernel(x, w_q, b_q, w_k, b_k, w_v, b_v, w_out, w_mask, b_mask, gamma):
    global LAST_RESULTS
    x = np.ascontiguousarray(np.asarray(x, dtype=np.float32))
    gamma_f = float(np.asarray(gamma).reshape(-1)[0])
    inv_gamma = float(1.0 / gamma_f) if gamma_f != 0.0 else float("inf")

    # fold spatial whitening (subtract channel-mean over P) into q/k weights
    C = np.eye(P, dtype=np.float64) - 1.0 / P
    wq = (C @ np.asarray(w_q, dtype=np.float64)).astype(np.float32)
    bq = (C @ np.asarray(b_q, dtype=np.float64)).astype(np.float32)
    wk = (C @ np.asarray(w_k, dtype=np.float64)).astype(np.float32)
    bk = (C @ np.asarray(b_k, dtype=np.float64)).astype(np.float32)

    wqk = np.concatenate([wq.T, wk.T], axis=1).astype(np.float16)  # [256, 256]
    wvm = np.concatenate(
        [np.asarray(w_v, np.float32).T, np.asarray(w_mask, np.float32).T],
        axis=1,
    ).astype(np.float16)                                            # [256, 129]
    bvm = np.concatenate(
        [np.asarray(b_v, np.float32).reshape(-1), np.zeros(1, np.float32)]
    ).reshape(1, 129)
    base = {
        "wqk": np.ascontiguousarray(wqk.reshape(2, 128, 256)),
        "wvm": np.ascontiguousarray(wvm.reshape(2, 128, 129)),
        "bqk": np.ascontiguousarray(np.stack([bq, bk], axis=1).astype(np.float32)),
        "bvm": np.ascontiguousarray(bvm),
        "woutT": np.ascontiguousarray(np.asarray(w_out, np.float32).T.astype(np.float16)),
    }
    xf = x.reshape(B, CIN, NQ).astype(np.float16)
    in_maps = [dict(base, x=np.ascontiguousarray(xf[c])) for c in range(N_CORES)]

    _maybe_shim_trace_hooks()
    nc = _build_bass(inv_gamma)
    res = run_bass_kernel_spmd(nc, in_maps, list(range(N_CORES)))
    LAST_RESULTS = res

    out = np.stack([res.results[c]["out"] for c in range(N_CORES)], axis=0)
    return out.reshape(B, CIN, H, W).astype(np.float32)
